# revision 33
# baseline (speedup 1.0000x reference)
"""LocalL1Loss Trainium2 kernel (8 NeuronCores, pure data parallel).

Reference semantics (KERNEL_SIZE=7):
    tp = zero-pad(targets, 3 on each spatial side)
    d_s = mean_c |inputs - shift_s(tp)|      for the 49 shifts s
    out = mean_{n,h,w} min_s d_s

Per core (2 of 16 batch items):
  - host: cast to bf16; zero-pad targets; pre-arrange into the exact SBUF
    layouts (128 partitions = 16x8 grid of 32x64 pixel patches, 3 channel
    planes per batch item per partition; targets carry a 3px halo -> 38x70
    per patch, stored twice with a one-element column offset so every column
    shift reads a 4-byte-aligned bf16 window -> VectorE 2x mode throughout).
  - per shift, the work is spread over FOUR engines (baseline had DVE at
    688us busy doing sub+2 adds+min while PE sat idle):
      DVE : bf16 2x subtract (3072 cyc) + final best update (1024 cyc)
      Abs : Act engine for ~57% of shifts; DVE tensor_scalar
            (bitwise_and 0x7FFF on uint16 view = bf16 abs) @4x for the rest
      PE  : channel sum as 3 accumulating identity matmuls per PSUM bank,
            plus a 4th (-identity x best) -> PSUM d = sum_c|diff| - best
      Act : r = Relu(-d) read straight from PSUM  (min(best,S) = best - r)
    engine-balance ~480us/engine vs 679us baseline.
  - epilogue: free-dim reduce_sum -> [128,1] fp32 partials, gpsimd
    partition_all_reduce -> scalar, single-descriptor DMA out; host sums the
    8 per-core scalars and divides by 3*N*H*W.
"""

import numpy as np
import ml_dtypes

import concourse.bacc as bacc
import concourse.mybir as mybir
from concourse import bass_isa, tile
from concourse.bass_utils import run_bass_kernel_spmd

# geometry (hardcoded for the [16, 3, 512, 512] problem)
B, C, H, W = 16, 3, 512, 512
K = 7
PAD = K // 2
NCORES = 8
BC = B // NCORES            # batch per core = 2
PH, PW = 16, 8              # patch grid -> 128 partitions
PR, PC = H // PH, W // PW   # 32 x 64 patch
HR, HC = PR + K - 1, PC + K - 1          # 38 x 70 halo patch
HP, WP = H + K - 1, W + K - 1 + 1        # padded target: 518 x 519 (+1 col)
NPIX = PR * PC              # 2048 pixels per partition per item
NCHK = 4                    # PSUM banks per shift: 4 x 512 fp32

BF16 = mybir.dt.bfloat16
U16 = mybir.dt.uint16
F32 = mybir.dt.float32

_CACHE = {}


def _steps():
    """(item, i, j) schedule: item0 even-j warmup while DMAs land, then
    item0/item1 interleaved so their serial best-update chains overlap."""
    p = [[(n, i, j) for i in range(K) for j in range(K) if j % 2 == par]
         for n in range(BC) for par in (0, 1)]
    p0e, p0o, p1e, p1o = p[0], p[1], p[2], p[3]

    def weave(a, b):
        out = []
        for x, y in zip(a, b):
            out += [x, y]
        out += a[len(b):] + b[len(a):]
        return out

    return weave(p0e + p0o, p1e + p1o)


def _build():
    nc = bacc.Bacc("TRN2", target_bir_lowering=False, debug=False,
                   num_devices=NCORES)

    x_d = [nc.dram_tensor(f"x{n}", [128, C, PR, PC], BF16,
                          kind="ExternalInput") for n in range(BC)]
    te_d = [nc.dram_tensor(f"te{n}", [128, C, HR, HC], BF16,
                           kind="ExternalInput") for n in range(BC)]
    to_d = [nc.dram_tensor(f"to{n}", [128, C, HR, HC], BF16,
                           kind="ExternalInput") for n in range(BC)]
    ip_d = nc.dram_tensor("identp", [128, 128], BF16, kind="ExternalInput")
    in_d = nc.dram_tensor("identn", [128, 128], BF16, kind="ExternalInput")
    out_d = nc.dram_tensor("out", [1, 1], F32, kind="ExternalOutput")

    steps = _steps()

    with tile.TileContext(nc) as tc:
        with (
            tc.tile_pool(name="persist", bufs=1) as pp,
            tc.tile_pool(name="work", bufs=4) as wp,
            tc.tile_pool(name="relu", bufs=3) as rp,
            tc.tile_pool(name="psum", bufs=2, space="PSUM") as qp,
        ):
            xt = [pp.tile([128, C, PR, PC], BF16, name=f"x{n}", tag=f"x{n}")
                  for n in range(BC)]
            te = [pp.tile([128, C, HR, HC], BF16, name=f"te{n}", tag=f"te{n}")
                  for n in range(BC)]
            to = [pp.tile([128, C, HR, HC], BF16, name=f"to{n}", tag=f"to{n}")
                  for n in range(BC)]
            best = [pp.tile([128, NPIX], BF16, name=f"best{n}", tag=f"best{n}")
                    for n in range(BC)]
            identp = pp.tile([128, 128], BF16, name="identp", tag="identp")
            identn = pp.tile([128, 128], BF16, name="identn", tag="identn")

            # loads in consumption order: identities + item0, item1, halos.
            # The startup-critical c0 tiles are split into 4 partition chunks
            # so they spread over parallel DMA queues (a single-queue 0.5 MB
            # transfer costs ~9us; the first subtract waits on it).
            for c in range(C):
                for n in range(BC):
                    for q in range(4):
                        lo, hi = q * 32, (q + 1) * 32
                        nc.sync.dma_start(out=xt[n][lo:hi, c],
                                          in_=x_d[n].ap()[lo:hi, c])
                        nc.sync.dma_start(out=te[n][lo:hi, c],
                                          in_=te_d[n].ap()[lo:hi, c])
                if c == 0:
                    # identities ride after the startup-critical c0 tiles
                    # (first needed by PE at ~19us, not before the first sub)
                    nc.sync.dma_start(out=identp[:], in_=ip_d.ap())
                    nc.sync.dma_start(out=identn[:], in_=in_d.ap())
            for n in range(BC):
                for c in range(C):
                    for q in range(2):
                        lo, hi = q * 64, (q + 1) * 64
                        nc.sync.dma_start(out=to[n][lo:hi, c],
                                          in_=to_d[n].ap()[lo:hi, c])

            rsum = pp.tile([128, 1], F32, tag="rsum")
            rtmp = pp.tile([128, 1], F32, tag="rtmp")
            rsc = pp.tile([128, 1], F32, tag="rsc")
            seen = [0] * BC
            for idx, (n, i, j) in enumerate(steps):
                tsel, joff = (te[n], j) if j % 2 == 0 else (to[n], j - 1)
                diff = wp.tile([128, C, PR, PC], BF16, tag="diff")
                if idx < 2:
                    # per-channel subs so compute starts after the first
                    # (x, te) channel pair lands, not the whole tiles
                    for c in range(C):
                        nc.vector.tensor_tensor(
                            out=diff[:, c], in0=xt[n][:, c],
                            in1=tsel[:, c, i:i + PR, joff:joff + PC],
                            op=mybir.AluOpType.subtract)
                else:
                    nc.vector.tensor_tensor(
                        out=diff[:], in0=xt[n][:],
                        in1=tsel[:, :, i:i + PR, joff:joff + PC],
                        op=mybir.AluOpType.subtract)
                ab = wp.tile([128, C, PR, PC], BF16, tag="ab")
                if idx < 2:
                    # warmup: per-channel DVE 4x Abs (Act idles during the
                    # ramp anyway; its quota is repaid at idx 4..13) so PE's
                    # chsum matmuls start as soon as each channel's DMA lands
                    for c in range(C):
                        nc.vector.tensor_scalar(
                            out=ab[:, c].bitcast(U16),
                            in0=diff[:, c].bitcast(U16),
                            scalar1=0x7FFF, scalar2=None,
                            op0=mybir.AluOpType.bitwise_and)
                elif idx < 4 or (idx % 5 in (1, 3) and idx >= 14):
                    # bf16 |x| = clear the sign bit; tensor_scalar runs in
                    # DVE 4x mode (1536 cyc) vs Act's 6144 @ 1.2GHz
                    nc.vector.tensor_scalar(
                        out=ab[:].bitcast(U16), in0=diff[:].bitcast(U16),
                        scalar1=0x7FFF, scalar2=None,
                        op0=mybir.AluOpType.bitwise_and)
                else:
                    nc.scalar.activation(
                        out=ab[:], in_=diff[:],
                        func=mybir.ActivationFunctionType.Abs)
                # PE channel sum: per 512-wide PSUM bank, 3 accumulating
                # identity matmuls (weights self-load, ~215ns each)
                abf = ab[:].rearrange("p c r w -> p c (r w)")
                S = qp.tile([128, NCHK, 512], F32, tag="S")
                for c in range(C):
                    for k in range(NCHK):
                        nc.tensor.matmul(
                            S[:, k], lhsT=identp[:],
                            rhs=abf[:, c, k * 512:(k + 1) * 512],
                            start=(c == 0),
                            stop=(c == C - 1 and seen[n] == 0))
                Sf = S[:].rearrange("p a b -> p (a b)")
                if seen[n] == 0:
                    # first shift for this batch item: best = S
                    nc.scalar.copy(out=best[n][:], in_=Sf)
                else:
                    # d = S - best on PE, r = relu(-d) on Act, then
                    # best -= r on DVE:  min(best, S) = best - relu(best-S)
                    for k in range(NCHK):
                        nc.tensor.matmul(
                            S[:, k], lhsT=identn[:],
                            rhs=best[n][:, k * 512:(k + 1) * 512],
                            start=False, stop=True)
                    r = rp.tile([128, NPIX], BF16, tag="r")
                    nc.scalar.activation(
                        out=r[:], in_=Sf,
                        func=mybir.ActivationFunctionType.Relu, scale=-1.0)
                    nc.vector.tensor_tensor(
                        out=best[n][:], in0=best[n][:], in1=r[:],
                        op=mybir.AluOpType.subtract)
                seen[n] += 1
                if seen[n] == K * K:
                    # item finished: fold its free-dim reduce in right away,
                    # overlapping the other item's remaining chain
                    nc.vector.tensor_reduce(
                        out=(rsum if n == 0 else rtmp)[:], in_=best[n][:],
                        axis=mybir.AxisListType.X, op=mybir.AluOpType.add)

            nc.vector.tensor_tensor(out=rsum[:], in0=rsum[:], in1=rtmp[:],
                                    op=mybir.AluOpType.add)
            # partition-reduce [128,1] -> scalar on gpsimd so the output
            # DMA is a single descriptor, not 128 four-byte ones (~7us)
            nc.gpsimd.partition_all_reduce(
                rsc[:], rsum[:], 128, bass_isa.ReduceOp.add)
            nc.sync.dma_start(out=out_d.ap(), in_=rsc[0:1, :])

    nc.compile()
    return nc


def _prep(inputs, targets):
    bf = ml_dtypes.bfloat16
    inputs = np.asarray(inputs, dtype=np.float32)
    targets = np.asarray(targets, dtype=np.float32)
    x_bf = inputs.astype(bf)
    tp = np.zeros((B, C, HP, WP), dtype=np.float32)
    tp[:, :, PAD:PAD + H, PAD:PAD + W] = targets
    t_bf = tp.astype(bf)
    identp = np.eye(128, dtype=np.float32).astype(bf)
    identn = (-np.eye(128, dtype=np.float32)).astype(bf)

    def halo(base):                       # base: [C, HP, >=WP-1] bf16 view
        s = base.strides
        v = np.lib.stride_tricks.as_strided(
            base, shape=(C, PH, PW, HR, HC),
            strides=(s[0], PR * s[1], PC * s[2], s[1], s[2]))
        return np.ascontiguousarray(v.transpose(1, 2, 0, 3, 4)
                                    .reshape(128, C, HR, HC))

    in_maps = []
    for core in range(NCORES):
        m = {"identp": identp, "identn": identn}
        for n in range(BC):
            b = core * BC + n
            m[f"x{n}"] = np.ascontiguousarray(
                x_bf[b].reshape(C, PH, PR, PW, PC)
                       .transpose(1, 3, 0, 2, 4).reshape(128, C, PR, PC))
            m[f"te{n}"] = halo(t_bf[b])
            m[f"to{n}"] = halo(t_bf[b, :, :, 1:])
        in_maps.append(m)
    return in_maps


def _run(inputs, targets, trace=False, **kw):
    if "nc" not in _CACHE:
        _CACHE["nc"] = _build()
    nc = _CACHE["nc"]
    in_maps = _prep(inputs, targets)
    res = run_bass_kernel_spmd(nc, in_maps, list(range(NCORES)),
                               trace=trace, **kw)
    total = 0.0
    for core in range(NCORES):
        total += res.results[core]["out"].astype(np.float64).sum()
    val = np.float32(total / (C * B * H * W))
    return np.asarray(val, dtype=np.float32), res


def kernel(inputs, targets):
    out, _ = _run(inputs, targets)
    return out
